# revision 6
# baseline (speedup 1.0000x reference)
"""GPTQ 4-bit quantized linear (CaiQuantLinear) on 8 TRN2 NeuronCores.

Computes out = x @ dequant(qweight, scales, qzeros) + bias where
  x: (4, 2048, 4096) fp16, qweight: (512, 4096) int32 (8x 4-bit per word,
  packed along input features), scales: (32, 4096) fp16, qzeros: (32, 512)
  int32 (packed along output features), bias: (4096,) fp16.
  Groups are contiguous blocks of 128 input features (g_idx = arange//128).

Sharding: 2-way over sequence x 4-way over output columns. Core c handles
seq rows (c//4)*4096..+4096 and output columns (c%4)*1024..+1024. Each
core sees half of x and a quarter of the weights; the host assembles the
2x4 grid. No collectives.

Per-core kernel (all-fp16 matmul, fp32 PSUM accumulate):
  1. Dequant with NO PE transposes: each qweight word-row is DMA-broadcast
     to 8 consecutive partitions (stride-0 middle dim in the source AP), so
     partition p of k-tile kt holds the word for input feature k=128*kt+p.
     A per-partition shift vector (iota & 31) extracts nibble p%8 straight
     into [k, out] layout. Group g == k-tile kt, so scale/zero are constant
     per partition-tile and are applied as two tensor_tensor ops against
     row-broadcast [128, out] tiles of s and -(z+1)*s.
  2. x streams through DMA-transpose (single HWDGE ring; two concurrent
     xbar rings corrupt data) into per-k [128, 512-seq] tiles.
  3. Chunk 0 runs k-outer (paced by the dequant pipeline); later chunks run
     bank-outer/k-inner so each PSUM bank finishes 32-k accumulation early
     and its drain (fp32 bias add -> fp16, SWDGE store) overlaps the next
     bank's matmuls. PE does nothing but back-to-back N=512 matmuls.
"""

import sys

if "/opt/trn_rl_repo" not in sys.path:
    sys.path.insert(0, "/opt/trn_rl_repo")

import numpy as np

B, S, IN, OUT = 4, 2048, 4096, 4096
SEQ = B * S                      # 8192
NCORES = 8
SEQ_SPLIT = 2
COL_SPLIT = 4
SEQ_S = SEQ // SEQ_SPLIT         # 4096 seq rows per core
OUT_S = OUT // COL_SPLIT         # 1024 output columns per core
PACK = 8                         # int32 packs 8 nibbles
GSIZE = 128                      # group size == k-tile size
CHUNK = 512

_CACHE = {}


def _build(seq, in_f, out_s, chunk):
    """Build + compile the per-core Bass program. All cores run the same
    NEFF on their own input slices (SPMD, no collectives)."""
    import concourse.bass as bass  # noqa: F401
    import concourse.mybir as mybir
    import concourse.tile as tile
    from concourse import bacc

    dt = mybir.dt
    op = mybir.AluOpType
    P = 128
    KT = in_f // P                # k-tiles (== groups) = 32
    QR = in_f // PACK             # qweight rows = 512
    WPT = P // PACK               # word-rows per k-tile = 16
    NCH = seq // chunk            # seq chunks = 8
    ST = chunk // P               # seq tiles per chunk = 4
    HB = out_s // 512             # 512-wide output halves = 2
    NB = ST * HB                  # PSUM banks per chunk = 8
    ZC = out_s // PACK            # qzeros columns = 128

    nc = bacc.Bacc("TRN2", target_bir_lowering=False, debug=False,
                   num_devices=NCORES)

    x_d = nc.dram_tensor("x", (seq, in_f), dt.float16, kind="ExternalInput")
    qw_d = nc.dram_tensor("qweight", (QR, out_s), dt.int32, kind="ExternalInput")
    sc_d = nc.dram_tensor("scales", (KT, out_s), dt.float16, kind="ExternalInput")
    qz_d = nc.dram_tensor("qzeros", (KT, ZC), dt.int32, kind="ExternalInput")
    b_d = nc.dram_tensor("bias", (1, out_s), dt.float16, kind="ExternalInput")
    out_d = nc.dram_tensor("out", (seq, out_s), dt.float16, kind="ExternalOutput")

    x = x_d.ap()
    qw = qw_d.ap()
    scales = sc_d.ap()
    qzeros = qz_d.ap()
    bias = b_d.ap()
    out = out_d.ap()

    with tile.TileContext(nc) as tc:
        with (
            tc.tile_pool(name="const", bufs=1) as const_pool,
            tc.tile_pool(name="w", bufs=1) as w_pool,
            tc.tile_pool(name="qb", bufs=4) as qb_pool,
            tc.tile_pool(name="wi", bufs=3) as wi_pool,
            tc.tile_pool(name="tmp", bufs=3) as tmp_pool,
            tc.tile_pool(name="zsb", bufs=4) as zsb_pool,
            tc.tile_pool(name="xt", bufs=66) as xt_pool,
            tc.tile_pool(name="ot", bufs=6) as out_pool,
            tc.tile_pool(name="ps", bufs=8, space="PSUM") as psum_pool,
            tc.tile_pool(name="dram", bufs=1, space="DRAM") as dram_pool,
        ):
            # ---- constants ----
            # per-partition nibble shift: 4*(p % 8)
            shv_raw = const_pool.tile([P, 1], dt.int32)
            nc.gpsimd.iota(shv_raw, pattern=[[0, 1]], base=0,
                           channel_multiplier=4)
            shv = const_pool.tile([P, 1], dt.int32)
            nc.vector.tensor_scalar(out=shv, in0=shv_raw, scalar1=0x1F,
                                    scalar2=None, op0=op.bitwise_and)

            bias16 = const_pool.tile([P, out_s], dt.float16)
            nc.gpsimd.dma_start(bias16, bias.to_broadcast((P, out_s)))
            bias32 = const_pool.tile([P, out_s], dt.float32)
            nc.vector.tensor_copy(bias32, bias16)

            # ---- scale / zero staging: zs_d[g] = [-(z+1)*s | s] ----
            s_sb = const_pool.tile([KT, out_s], dt.float16)
            nc.gpsimd.dma_start(s_sb, scales)
            qz_sb = const_pool.tile([KT, ZC], dt.int32)
            nc.gpsimd.dma_start(qz_sb, qzeros)
            z_i = const_pool.tile([KT, out_s], dt.int32)
            z_iv = z_i.rearrange("g (c s) -> g c s", s=PACK)
            for s in range(PACK):
                nc.vector.tensor_scalar(
                    out=z_iv[:, :, s], in0=qz_sb, scalar1=4 * s, scalar2=0xF,
                    op0=op.logical_shift_right, op1=op.bitwise_and)
            z1_16 = const_pool.tile([KT, out_s], dt.float16)
            nc.vector.tensor_scalar_add(z1_16, z_i, 1.0)
            zs_stage = const_pool.tile([KT, 2 * out_s], dt.float16)
            # -(z+1)*s  ==  (z1 * -1) * s
            nc.vector.scalar_tensor_tensor(
                out=zs_stage[:, 0:out_s], in0=z1_16, scalar=-1.0, in1=s_sb,
                op0=op.mult, op1=op.mult)
            nc.vector.tensor_copy(zs_stage[:, out_s:2 * out_s], s_sb)
            zs_d = dram_pool.tile([KT, 2 * out_s], dt.float16)
            nc.gpsimd.dma_start(zs_d, zs_stage)

            # ---- chunk-0 x transposes (sync ring only) ----
            def issue_transposes(cn):
                xts = []
                for k in range(KT):
                    xtk = xt_pool.tile([P, chunk], dt.float16, tag="xt",
                                       name=f"xt{cn}_{k}")
                    nc.sync.dma_start(
                        xtk,
                        x[cn * chunk:(cn + 1) * chunk, k * P:(k + 1) * P],
                        transpose=True)
                    xts.append(xtk)
                return xts

            xts0 = issue_transposes(0)

            # ---- dequant (no PE involvement), interleaved with chunk-0
            # k-outer matmuls so the PE starts as soon as k-tile 0 is ready.
            w_all = w_pool.tile([P, KT, out_s], dt.float16)
            pss = [psum_pool.tile([P, 512], dt.float32, tag="acc",
                                  name=f"ps0_{b}") for b in range(NB)]
            for kt in range(KT):
                qb = qb_pool.tile([P, out_s], dt.int32, tag="qb")
                src = qw[kt * WPT:(kt + 1) * WPT, :].unsqueeze(1) \
                    .broadcast_to((WPT, PACK, out_s))
                nc.scalar.dma_start(qb, src)
                wi32 = wi_pool.tile([P, out_s], dt.int32, tag="wi")
                nc.vector.tensor_scalar(
                    out=wi32, in0=qb, scalar1=shv, scalar2=0xF,
                    op0=op.logical_shift_right, op1=op.bitwise_and)
                zsb = zsb_pool.tile([P, 2 * out_s], dt.float16, tag="zsb")
                nc.gpsimd.dma_start(
                    zsb, zs_d[kt:kt + 1, :].to_broadcast((P, 2 * out_s)))
                tmp16 = tmp_pool.tile([P, out_s], dt.float16, tag="tmp")
                nc.vector.tensor_tensor(
                    out=tmp16, in0=wi32, in1=zsb[:, out_s:2 * out_s],
                    op=op.mult)
                nc.vector.tensor_tensor(
                    out=w_all[:, kt, :], in0=tmp16, in1=zsb[:, 0:out_s],
                    op=op.add)
                # chunk-0 matmuls for this k-tile (k-outer, bank-inner)
                for b in range(NB):
                    st, hb = b % ST, b // ST
                    nc.tensor.matmul(
                        pss[b],
                        lhsT=xts0[kt][:, st * P:(st + 1) * P],
                        rhs=w_all[:, kt, hb * 512:(hb + 1) * 512],
                        start=(kt == 0), stop=(kt == KT - 1))

            # chunk-1 transposes can start while chunk-0 computes
            xts_next = issue_transposes(1)

            # chunk-0 drains
            for b in range(NB):
                st, hb = b % ST, b // ST
                o16 = out_pool.tile([P, 512], dt.float16, tag="o16")
                nc.vector.tensor_add(o16, pss[b],
                                     bias32[:, hb * 512:(hb + 1) * 512])
                nc.gpsimd.dma_start(
                    out[st * P:(st + 1) * P, hb * 512:(hb + 1) * 512], o16)

            # ---- main loop: bank-outer, k-inner; drain each bank early ----
            for cn in range(1, NCH):
                xts = xts_next
                if cn + 1 < NCH:
                    xts_next = issue_transposes(cn + 1)
                pss = [psum_pool.tile([P, 512], dt.float32, tag="acc",
                                      name=f"ps{cn}_{b}") for b in range(NB)]
                for b in range(NB):
                    st, hb = b % ST, b // ST
                    for k in range(KT):
                        nc.tensor.matmul(
                            pss[b],
                            lhsT=xts[k][:, st * P:(st + 1) * P],
                            rhs=w_all[:, k, hb * 512:(hb + 1) * 512],
                            start=(k == 0), stop=(k == KT - 1))
                    o16 = out_pool.tile([P, 512], dt.float16, tag="o16")
                    nc.vector.tensor_add(o16, pss[b],
                                         bias32[:, hb * 512:(hb + 1) * 512])
                    r0 = cn * chunk + st * P
                    nc.gpsimd.dma_start(
                        out[r0:r0 + P, hb * 512:(hb + 1) * 512], o16)

    nc.compile()
    return nc


def _get_program(seq, in_f, out_s, chunk):
    key = (seq, in_f, out_s, chunk)
    if key not in _CACHE:
        _CACHE[key] = _build(seq, in_f, out_s, chunk)
    return _CACHE[key]


def make_in_maps(x2, qweight, scales, qzeros, bias):
    """Per-core input slices for the 2-way-seq x 4-way-col sharding."""
    zc = OUT_S // PACK
    in_maps = []
    for c in range(NCORES):
        h, q = divmod(c, COL_SPLIT)
        o0 = q * OUT_S
        in_maps.append({
            "x": np.ascontiguousarray(x2[h * SEQ_S:(h + 1) * SEQ_S, :]),
            "qweight": np.ascontiguousarray(qweight[:, o0:o0 + OUT_S]),
            "scales": np.ascontiguousarray(scales[:, o0:o0 + OUT_S]),
            "qzeros": np.ascontiguousarray(qzeros[:, q * zc:(q + 1) * zc]),
            "bias": np.ascontiguousarray(bias[o0:o0 + OUT_S].reshape(1, OUT_S)),
        })
    return in_maps


def assemble(results):
    """Stitch the 2x4 per-core output grid back to (B, S, OUT) fp16."""
    full = np.empty((SEQ, OUT), dtype=np.float16)
    for c in range(NCORES):
        h, q = divmod(c, COL_SPLIT)
        full[h * SEQ_S:(h + 1) * SEQ_S, q * OUT_S:(q + 1) * OUT_S] = \
            results[c]["out"]
    return full.reshape(B, S, OUT)


def kernel(x, qweight, scales, qzeros, g_idx=None, bias=None, **_unused):
    """Full-input entry point: shards over 8 cores, runs on HW, gathers."""
    from concourse.bass_utils import run_bass_kernel_spmd

    x = np.asarray(x)
    qweight = np.asarray(qweight)
    scales = np.asarray(scales)
    qzeros = np.asarray(qzeros)
    bias = np.asarray(bias)

    x2 = np.ascontiguousarray(x.reshape(SEQ, IN))
    nc = _get_program(SEQ_S, IN, OUT_S, CHUNK)
    in_maps = make_in_maps(x2, qweight, scales, qzeros, bias)
    res = run_bass_kernel_spmd(nc, in_maps, core_ids=list(range(NCORES)))
    return assemble(res.results).astype(np.float16)


# revision 11
# speedup vs baseline: 1.1595x; 1.1595x over previous
"""GPTQ 4-bit quantized linear (CaiQuantLinear) on 8 TRN2 NeuronCores.

Computes out = x @ dequant(qweight, scales, qzeros) + bias where
  x: (4, 2048, 4096) fp16, qweight: (512, 4096) int32 (8x 4-bit per word,
  packed along input features), scales: (32, 4096) fp16, qzeros: (32, 512)
  int32 (packed along output features), bias: (4096,) fp16.
  Groups are contiguous blocks of 128 input features (g_idx = arange//128).

Sharding: 2-way over sequence x 4-way over output columns. Core c handles
seq rows (c//4)*4096..+4096 and output columns (c%4)*1024..+1024; the host
assembles the 2x4 grid. No collectives. Halving x per core halves the
DMA-transpose chain (the scarce resource: one transpose ring at ~1.3us per
[1024,128] tile regardless of smaller sizes; all DMAs share one ~17-deep
completion window, so DMA COUNT is the currency).

Per-core kernel (fp16 matmul, fp32 PSUM accumulate; PE does only matmuls):
  1. Dequant with no PE involvement: each qweight word-row is DMA-broadcast
     to 8 consecutive partitions (stride-0 middle dim in the source AP), so
     partition p of k-tile kt holds the word for input feature k=128*kt+p.
     A per-partition shift vector (iota & 31) extracts nibble p%8 straight
     into [k, out] layout (gpsimd). Group g == k-tile kt, so scale/zero are
     constant per tile and applied as two vector tensor_tensor ops against
     row-broadcast tiles of [-(z+1)*s | s] (staged to DRAM once, then
     row-broadcast per group).
  2. x streams through DMA-transpose on the sync ring only (two concurrent
     xbar rings corrupt data) into per-k [128, 1024-seq] tiles.
  3. Chunk 0 pass A is k-outer, paced by the dequant pipeline; all other
     passes run 256 back-to-back N=512 matmuls per (chunk, column-half)
     with 8 PSUM banks, draining (fp32 bias add -> fp16, SWDGE store)
     between passes. Next-chunk transposes are issued one chunk ahead.
"""

import sys

if "/opt/trn_rl_repo" not in sys.path:
    sys.path.insert(0, "/opt/trn_rl_repo")

import numpy as np

B, S, IN, OUT = 4, 2048, 4096, 4096
SEQ = B * S                      # 8192
NCORES = 8
SEQ_SPLIT = 2
COL_SPLIT = 4
SEQ_S = SEQ // SEQ_SPLIT         # 4096 seq rows per core
OUT_S = OUT // COL_SPLIT         # 1024 output columns per core
PACK = 8                         # int32 packs 8 nibbles
GSIZE = 128                      # group size == k-tile size
CHUNK = 1024

_CACHE = {}


def _build(seq, in_f, out_s, chunk):
    """Build + compile the per-core Bass program. All cores run the same
    NEFF on their own input slices (SPMD, no collectives)."""
    import concourse.bass as bass  # noqa: F401
    import concourse.mybir as mybir
    import concourse.tile as tile
    from concourse import bacc

    dt = mybir.dt
    op = mybir.AluOpType
    P = 128
    KT = in_f // P                # k-tiles (== groups) = 32
    QR = in_f // PACK             # qweight rows = 512
    WPT = P // PACK               # word-rows per k-tile = 16
    NCH = seq // chunk            # seq chunks = 4
    ST = chunk // P               # seq tiles per chunk = 8
    HB = out_s // 512             # 512-wide output halves = 2
    ZC = out_s // PACK            # qzeros columns = 128

    nc = bacc.Bacc("TRN2", target_bir_lowering=False, debug=False,
                   num_devices=NCORES)

    x_d = nc.dram_tensor("x", (seq, in_f), dt.float16, kind="ExternalInput")
    qw_d = nc.dram_tensor("qweight", (QR, out_s), dt.int32, kind="ExternalInput")
    sc_d = nc.dram_tensor("scales", (KT, out_s), dt.float16, kind="ExternalInput")
    qz_d = nc.dram_tensor("qzeros", (KT, ZC), dt.int32, kind="ExternalInput")
    b_d = nc.dram_tensor("bias", (1, out_s), dt.float16, kind="ExternalInput")
    out_d = nc.dram_tensor("out", (seq, out_s), dt.float16, kind="ExternalOutput")

    x = x_d.ap()
    qw = qw_d.ap()
    scales = sc_d.ap()
    qzeros = qz_d.ap()
    bias = b_d.ap()
    out = out_d.ap()

    with tile.TileContext(nc) as tc:
        with (
            tc.tile_pool(name="const", bufs=1) as const_pool,
            tc.tile_pool(name="w", bufs=1) as w_pool,
            tc.tile_pool(name="qb", bufs=2) as qb_pool,
            tc.tile_pool(name="wi", bufs=2) as wi_pool,
            tc.tile_pool(name="wi16", bufs=2) as wi16_pool,
            tc.tile_pool(name="tmp", bufs=2) as tmp_pool,
            tc.tile_pool(name="zsb", bufs=2) as zsb_pool,
            tc.tile_pool(name="xt", bufs=38) as xt_pool,
            tc.tile_pool(name="ot", bufs=6) as out_pool,
            tc.tile_pool(name="ps", bufs=8, space="PSUM") as psum_pool,
            tc.tile_pool(name="dram", bufs=1, space="DRAM") as dram_pool,
        ):
            # ---- scale / zero staging FIRST (its DMAs must clear the
            # window before the transpose flood): zs_d[g] = [-(z+1)*s | s]
            s_sb = const_pool.tile([KT, out_s], dt.float16)
            nc.gpsimd.dma_start(s_sb, scales)
            qz_sb = const_pool.tile([KT, ZC], dt.int32)
            nc.gpsimd.dma_start(qz_sb, qzeros)
            bias16 = const_pool.tile([P, out_s], dt.float16)
            nc.gpsimd.dma_start(bias16, bias.to_broadcast((P, out_s)))
            bias32 = const_pool.tile([P, out_s], dt.float32)
            nc.vector.tensor_copy(bias32, bias16)

            # per-partition nibble shift: 4*(p % 8)
            shv_raw = const_pool.tile([P, 1], dt.int32)
            nc.gpsimd.iota(shv_raw, pattern=[[0, 1]], base=0,
                           channel_multiplier=4)
            shv = const_pool.tile([P, 1], dt.int32)
            nc.vector.tensor_scalar(out=shv, in0=shv_raw, scalar1=0x1F,
                                    scalar2=None, op0=op.bitwise_and)

            z_i = const_pool.tile([KT, out_s], dt.int32)
            z_iv = z_i.rearrange("g (c s) -> g c s", s=PACK)
            for s in range(PACK):
                nc.vector.tensor_scalar(
                    out=z_iv[:, :, s], in0=qz_sb, scalar1=4 * s, scalar2=0xF,
                    op0=op.logical_shift_right, op1=op.bitwise_and)
            z1_16 = const_pool.tile([KT, out_s], dt.float16)
            nc.vector.tensor_scalar_add(z1_16, z_i, 1.0)
            zs_stage = const_pool.tile([KT, 2 * out_s], dt.float16)
            # -(z+1)*s  ==  (z1 * -1) * s
            nc.vector.scalar_tensor_tensor(
                out=zs_stage[:, 0:out_s], in0=z1_16, scalar=-1.0, in1=s_sb,
                op0=op.mult, op1=op.mult)
            nc.vector.tensor_copy(zs_stage[:, out_s:2 * out_s], s_sb)
            zs_d = dram_pool.tile([KT, 2 * out_s], dt.float16)
            nc.gpsimd.dma_start(zs_d, zs_stage)

            # ---- x transposes (sync ring only) ----
            def issue_transposes(cn):
                xts = []
                for k in range(KT):
                    xtk = xt_pool.tile([P, chunk], dt.float16, tag="xt",
                                       name=f"xt{cn}_{k}")
                    nc.sync.dma_start(
                        xtk,
                        x[cn * chunk:(cn + 1) * chunk, k * P:(k + 1) * P],
                        transpose=True)
                    xts.append(xtk)
                return xts

            xts0 = issue_transposes(0)

            # ---- dequant pipeline, interleaved with chunk-0 pass-A
            # (cols 0:512) k-outer matmuls so the PE starts immediately.
            w_all = w_pool.tile([P, KT, out_s], dt.float16)
            pss = [psum_pool.tile([P, 512], dt.float32, tag="acc",
                                  name=f"ps0a_{b}") for b in range(ST)]
            for kt in range(KT):
                qb = qb_pool.tile([P, out_s], dt.int32, tag="qb")
                src = qw[kt * WPT:(kt + 1) * WPT, :].unsqueeze(1) \
                    .broadcast_to((WPT, PACK, out_s))
                nc.scalar.dma_start(qb, src)
                wi32 = wi_pool.tile([P, out_s], dt.int32, tag="wi")
                nc.vector.tensor_scalar(
                    out=wi32, in0=qb, scalar1=shv, scalar2=0xF,
                    op0=op.logical_shift_right, op1=op.bitwise_and)
                wi16 = wi16_pool.tile([P, out_s], dt.float16, tag="wi16")
                nc.scalar.copy(wi16, wi32)
                zsb = zsb_pool.tile([P, 2 * out_s], dt.float16, tag="zsb")
                nc.gpsimd.dma_start(
                    zsb, zs_d[kt:kt + 1, :].to_broadcast((P, 2 * out_s)))
                tmp16 = tmp_pool.tile([P, out_s], dt.float16, tag="tmp")
                nc.gpsimd.tensor_tensor(
                    out=tmp16, in0=wi16, in1=zsb[:, out_s:2 * out_s],
                    op=op.mult)
                nc.vector.tensor_tensor(
                    out=w_all[:, kt, :], in0=tmp16, in1=zsb[:, 0:out_s],
                    op=op.add)
                for st in range(ST):
                    nc.tensor.matmul(
                        pss[st],
                        lhsT=xts0[kt][:, st * P:(st + 1) * P],
                        rhs=w_all[:, kt, 0:512],
                        start=(kt == 0), stop=(kt == KT - 1))

            xts_next = issue_transposes(1)

            def drain(pss_list, cn, hb):
                for st, ps in enumerate(pss_list):
                    o16 = out_pool.tile([P, 512], dt.float16, tag="o16")
                    nc.vector.tensor_add(o16, ps,
                                         bias32[:, hb * 512:(hb + 1) * 512])
                    r0 = cn * chunk + st * P
                    nc.gpsimd.dma_start(
                        out[r0:r0 + P, hb * 512:(hb + 1) * 512], o16)

            drain(pss, 0, 0)

            def mm_pass(xts, cn, hb):
                pss = [psum_pool.tile([P, 512], dt.float32, tag="acc",
                                      name=f"ps{cn}_{hb}_{b}")
                       for b in range(ST)]
                for k in range(KT):
                    for st in range(ST):
                        nc.tensor.matmul(
                            pss[st],
                            lhsT=xts[k][:, st * P:(st + 1) * P],
                            rhs=w_all[:, k, hb * 512:(hb + 1) * 512],
                            start=(k == 0), stop=(k == KT - 1))
                drain(pss, cn, hb)

            # chunk-0 pass B (cols 512:1024) at full PE speed
            mm_pass(xts0, 0, 1)

            # ---- main loop ----
            for cn in range(1, NCH):
                xts = xts_next
                if cn + 1 < NCH:
                    xts_next = issue_transposes(cn + 1)
                for hb in range(HB):
                    mm_pass(xts, cn, hb)

    nc.compile()
    return nc


def _get_program(seq, in_f, out_s, chunk):
    key = (seq, in_f, out_s, chunk)
    if key not in _CACHE:
        _CACHE[key] = _build(seq, in_f, out_s, chunk)
    return _CACHE[key]


def make_in_maps(x2, qweight, scales, qzeros, bias):
    """Per-core input slices for the 2-way-seq x 4-way-col sharding."""
    zc = OUT_S // PACK
    in_maps = []
    for c in range(NCORES):
        h, q = divmod(c, COL_SPLIT)
        o0 = q * OUT_S
        in_maps.append({
            "x": np.ascontiguousarray(x2[h * SEQ_S:(h + 1) * SEQ_S, :]),
            "qweight": np.ascontiguousarray(qweight[:, o0:o0 + OUT_S]),
            "scales": np.ascontiguousarray(scales[:, o0:o0 + OUT_S]),
            "qzeros": np.ascontiguousarray(qzeros[:, q * zc:(q + 1) * zc]),
            "bias": np.ascontiguousarray(bias[o0:o0 + OUT_S].reshape(1, OUT_S)),
        })
    return in_maps


def assemble(results):
    """Stitch the 2x4 per-core output grid back to (B, S, OUT) fp16."""
    full = np.empty((SEQ, OUT), dtype=np.float16)
    for c in range(NCORES):
        h, q = divmod(c, COL_SPLIT)
        full[h * SEQ_S:(h + 1) * SEQ_S, q * OUT_S:(q + 1) * OUT_S] = \
            results[c]["out"]
    return full.reshape(B, S, OUT)


def kernel(x, qweight, scales, qzeros, g_idx=None, bias=None, **_unused):
    """Full-input entry point: shards over 8 cores, runs on HW, gathers."""
    from concourse.bass_utils import run_bass_kernel_spmd

    x = np.asarray(x)
    qweight = np.asarray(qweight)
    scales = np.asarray(scales)
    qzeros = np.asarray(qzeros)
    bias = np.asarray(bias)

    x2 = np.ascontiguousarray(x.reshape(SEQ, IN))
    nc = _get_program(SEQ_S, IN, OUT_S, CHUNK)
    in_maps = make_in_maps(x2, qweight, scales, qzeros, bias)
    res = run_bass_kernel_spmd(nc, in_maps, core_ids=list(range(NCORES)))
    return assemble(res.results).astype(np.float16)
